# revision 1
# baseline (speedup 1.0000x reference)
"""NonMaxSuppression (5x5 local max, thr=0) on 8 trn2 NeuronCores.

Input : scores [8, 1, 2048, 2048] fp32 (full).
Output: [2, 2_000_000] int32 — (h, w) coords of survivors in global
        row-major order, padded with -1 (matches jnp.nonzero(size=...)).

Data-parallel: image b -> core b. Each core computes the dense survivor
mask for its image on-device; the host unshards (nonzero + concat + pad).

Device algorithm (per 256-col strip, slab layout, all fp32-exact):
  partitions = 16-row slabs (+2 halo rows each side, zero-padded);
  H pass: 5-max along cols via pair pyramid (P1 = pairwise max,
          R5 = window-5 max from P1 + one raw col), relu folded in
          via scalar_tensor_tensor (max with 0).
  V pass: same pairs trick along rows of R5 (row shifts = free-dim
          offsets in slab layout).
  mask  = (x >= max(M5, TINY)) — exact: M5 >= x always, so x >= M5
          iff x == M5; TINY (1e-38) rejects x <= 0 (scores are randn,
          |x| >> 1e-38 whenever x > 0).
"""
import sys

sys.path.insert(0, "/opt/trn_rl_repo")
import numpy as np

import concourse.bass as bass
from concourse import mybir
from concourse.bass_utils import run_bass_kernel_spmd

B, H, W = 8, 2048, 2048
NCORES = 8
MAX_KEYPOINTS = 2_000_000
TINY = 1e-38  # smallest normal fp32 territory; > 0, < any positive score

ROWS = 16          # image rows per partition (128 * 16 = 2048)
FR = ROWS + 4      # frame rows incl. 2-row halo each side
WT = 256           # strip width
FC = WT + 4        # frame cols incl. 2-col halo
NSTRIP = W // WT

f32 = mybir.dt.float32
u8 = mybir.dt.uint8


def _dram_ap(t, offset, pattern):
    return bass.AP(tensor=t, offset=offset, ap=pattern)


def _build():
    nc = bass.Bass()
    x_in = nc.declare_dram_parameter("scores", [H, W], f32, isOutput=False)
    m_out = nc.declare_dram_parameter("mask", [H, W], u8, isOutput=True)

    with (
        nc.sbuf_tensor("xb0", [128, FR, FC], f32) as xb0,
        nc.sbuf_tensor("xb1", [128, FR, FC], f32) as xb1,
        nc.sbuf_tensor("xb2", [128, FR, FC], f32) as xb2,
        nc.sbuf_tensor("p1", [128, FR, FC // 2], f32) as p1,
        nc.sbuf_tensor("tt", [128, FR, WT // 2 + 1], f32) as tt,
        nc.sbuf_tensor("r5", [128, FR, WT], f32) as r5,
        nc.sbuf_tensor("p2", [128, FR // 2, WT], f32) as p2,
        nc.sbuf_tensor("t3", [128, ROWS // 2 + 1, WT], f32) as t3,
        nc.sbuf_tensor("m5", [128, ROWS, WT], f32) as m5,
        nc.sbuf_tensor("msk", [128, ROWS, W], u8) as msk,
        nc.Block() as block,
        nc.semaphore("load_sem") as load_sem,
        nc.semaphore("init_sem") as init_sem,
        nc.semaphore("free_sem") as free_sem,
        nc.semaphore("out_sem") as out_sem,
    ):
        bufs = [xb0, xb1, xb2]
        buf_of = lambda s: bufs[2] if s == NSTRIP - 1 else bufs[s % 2]

        def strip_src(s):
            # frame col f = image col WT*s - 2 + f
            c0 = WT * s - 2
            dc = max(0, -c0)           # dst col offset
            c0 = max(0, c0)
            c1 = min(W, WT * s - 2 + FC)
            return c0, dc, c1 - c0

        @block.sync
        def _(sync):
            sync.wait_ge(init_sem, 1)
            for s in range(NSTRIP):
                xb = buf_of(s)
                if s >= 2 and s != NSTRIP - 1:
                    sync.wait_ge(free_sem, s - 1)
                c0, dc, cw = strip_src(s)
                # partitions 1..126: rows 16p-2 .. 16p+17
                sync.dma_start(
                    out=xb[1:127, :, dc : dc + cw],
                    in_=_dram_ap(
                        x_in, 14 * W + c0, [[16 * W, 126], [W, FR], [1, cw]]
                    ),
                ).then_inc(load_sem, 16)
                # partition 0: rows 0..17 -> frame rows 2..19
                sync.dma_start(
                    out=xb[0:1, 2:FR, dc : dc + cw],
                    in_=_dram_ap(x_in, c0, [[0, 1], [W, FR - 2], [1, cw]]),
                ).then_inc(load_sem, 16)
                # partition 127: rows 2030..2047 -> frame rows 0..17
                sync.dma_start(
                    out=xb[127:128, 0 : FR - 2, dc : dc + cw],
                    in_=_dram_ap(
                        x_in, 2030 * W + c0, [[0, 1], [W, FR - 2], [1, cw]]
                    ),
                ).then_inc(load_sem, 16)

        @block.vector
        def _(v):
            A = mybir.AluOpType
            for s in range(NSTRIP):
                xb = buf_of(s)
                v.wait_ge(load_sem, 48 * (s + 1))
                # --- H pass ---
                v.scalar_tensor_tensor(
                    out=p1[:, :, :], in0=xb[:, :, 0:FC:2], scalar=0.0,
                    in1=xb[:, :, 1:FC:2], op0=A.max, op1=A.max,
                )
                # Tall[k] = max(P1[k], P1[k+1]); Te = Tall[:-1], To = Tall[1:]
                v.tensor_tensor(
                    out=tt[:, :, :], in0=p1[:, :, 0:129], in1=p1[:, :, 1:130],
                    op=A.max,
                )
                v.scalar_tensor_tensor(
                    out=r5[:, :, 0:WT:2], in0=xb[:, :, 4:FC:2], scalar=0.0,
                    in1=tt[:, :, 0:128], op0=A.max, op1=A.max,
                )
                v.scalar_tensor_tensor(
                    out=r5[:, :, 1:WT:2], in0=xb[:, :, 1 : FC - 4 : 2],
                    scalar=0.0, in1=tt[:, :, 1:129], op0=A.max, op1=A.max,
                )
                # --- V pass ---
                v.tensor_tensor(
                    out=p2[:, :, :], in0=r5[:, 0:FR:2, :], in1=r5[:, 1:FR:2, :],
                    op=A.max,
                )
                # T3[i] = max(P2[i], P2[i+1]); even rows use T3[:-1], odd T3[1:]
                v.tensor_tensor(
                    out=t3[:, :, :], in0=p2[:, 0:9, :], in1=p2[:, 1:10, :],
                    op=A.max,
                )
                v.scalar_tensor_tensor(
                    out=m5[:, 0:ROWS:2, :], in0=r5[:, 4:FR:2, :], scalar=TINY,
                    in1=t3[:, 0:8, :], op0=A.max, op1=A.max,
                )
                v.scalar_tensor_tensor(
                    out=m5[:, 1:ROWS:2, :], in0=r5[:, 1:ROWS:2, :], scalar=TINY,
                    in1=t3[:, 1:9, :], op0=A.max, op1=A.max,
                )
                # --- mask ---
                v.tensor_tensor(
                    out=msk[:, :, WT * s : WT * (s + 1)],
                    in0=xb[:, 2 : 2 + ROWS, 2 : 2 + WT], in1=m5[:, :, :],
                    op=A.is_ge,
                )
                v.drain().then_inc(free_sem, 1)

        @block.gpsimd
        def _(g):
            # one-time zero of halo regions never written by loads; wide
            # partition ranges keep accesses quadrant-aligned — the extra
            # partitions' copies are overwritten by every strip load
            for xb in bufs:
                g.memset(xb[0:16, 0:2, :], 0.0)          # p0 top halo rows
                g.memset(xb[96:128, FR - 2 : FR, :], 0.0)  # p127 bottom halo
            g.memset(bufs[0][:, :, 0:2], 0.0)            # strip 0 left halo
            g.memset(bufs[2][:, :, FC - 2 : FC], 0.0)    # last strip right
            g.drain().then_inc(init_sem, 1)

        @block.scalar
        def _(sc):
            # stream mask strips out as compute finishes them
            for s in range(NSTRIP):
                sc.wait_ge(free_sem, s + 1)
                sc.dma_start(
                    out=_dram_ap(
                        m_out, WT * s, [[16 * W, 128], [W, ROWS], [1, WT]]
                    ),
                    in_=msk[:, :, WT * s : WT * (s + 1)],
                ).then_inc(out_sem, 16)
            sc.wait_ge(out_sem, 16 * NSTRIP)

    return nc


_nc = None


def kernel(scores: np.ndarray) -> np.ndarray:
    global _nc
    scores = np.ascontiguousarray(np.asarray(scores), dtype=np.float32)
    assert scores.shape == (B, 1, H, W), scores.shape
    if _nc is None:
        _nc = _build()
    in_maps = [
        {"scores": np.ascontiguousarray(scores[b, 0])} for b in range(NCORES)
    ]
    res = run_bass_kernel_spmd(_nc, in_maps, list(range(NCORES)), trace=False)
    hs, ws = [], []
    for b in range(NCORES):
        mask = res.results[b]["mask"]
        idx = np.flatnonzero(mask)  # row-major == (h, w) lexicographic
        hs.append((idx // W).astype(np.int32))
        ws.append((idx % W).astype(np.int32))
    hh = np.concatenate(hs)
    ww = np.concatenate(ws)
    n = min(len(hh), MAX_KEYPOINTS)
    out = np.full((2, MAX_KEYPOINTS), -1, dtype=np.int32)
    out[0, :n] = hh[:n]
    out[1, :n] = ww[:n]
    return out


if __name__ == "__main__":
    rng = np.random.default_rng(0)
    x = rng.standard_normal((B, 1, H, W), dtype=np.float32)
    out = kernel(scores=x)
    print("out", out.shape, out.dtype, "nvalid:", int((out[0] >= 0).sum()))



# revision 11
# speedup vs baseline: 1.9643x; 1.9643x over previous
"""NonMaxSuppression (5x5 local max, thr=0) on 8 trn2 NeuronCores.

Input : scores [8, 1, 2048, 2048] fp32 (full).
Output: [2, 2_000_000] int32 — (h, w) coords of survivors in global
        row-major order, padded with -1 (matches jnp.nonzero(size=...)).

Data-parallel: image b -> core b. Each core computes the dense survivor
mask for its image on-device; the host unshards (nonzero + concat + pad).

Device algorithm (per 256-col strip, slab layout, all fp32-exact):
  partitions = 16-row slabs (+2 halo rows each side, zero-padded);
  V pass first (frame rows 20 -> slab rows 16), then H pass — this
  order runs the 5-max over the row-halo'd dimension first (~8% fewer
  elem-ops than H-first). Pyramid per direction: P = pairwise max,
  T = adjacent pair maxes (4-windows), 5-max = T + one raw elem.
  TINY (1e-38) is folded into the T stage of the V pass via
  scalar_tensor_tensor, so m5 >= TINY everywhere and the final
  is_ge(x, m5) rejects x <= 0 exactly (scores are randn; any positive
  score >> TINY). No relu needed: zero-padded halos only ever lose
  against positive centers, and non-positive centers are rejected by
  the TINY floor.
  mask = (x >= m5) — exact: m5 >= x always, so x >= m5 iff x == m5.

All 5-max work runs on the Vector engine: on this toolchain the Pool
(GpSimd) engine's TensorTensor ucode only implements add/mult — no
max — so the pyramid cannot be split across engines.
"""
import sys

sys.path.insert(0, "/opt/trn_rl_repo")
import numpy as np

import concourse.bass as bass
from concourse import mybir
from concourse.bass_utils import run_bass_kernel_spmd

B, H, W = 8, 2048, 2048
NCORES = 8
MAX_KEYPOINTS = 2_000_000
TINY = 1e-38  # smallest normal fp32 territory; > 0, < any positive score

ROWS = 16          # image rows per partition (128 * 16 = 2048)
FR = ROWS + 4      # frame rows incl. 2-row halo each side
WT = 256           # strip width
FC = WT + 4        # frame cols incl. 2-col halo
NSTRIP = W // WT

f32 = mybir.dt.float32
u8 = mybir.dt.uint8


def _dram_ap(t, offset, pattern):
    return bass.AP(tensor=t, offset=offset, ap=pattern)


def _emit_nms(e, xb, t, msk, s):
    """V+H+compare pipeline for one strip on one engine."""
    A = mybir.AluOpType
    p2, t3, c5, p1, tt, m5e, m5o = (
        t["p2"], t["t3"], t["c5"], t["p1"], t["tt"], t["m5e"], t["m5o"]
    )
    hp = FC // 2       # p1 cols (130)
    ne = WT // 2       # outputs per parity (128)
    # --- V pass: 5-max down rows (frame rows 20 -> slab rows 16) ---
    e.tensor_tensor(
        out=p2[:, :, :], in0=xb[:, 0:FR:2, :], in1=xb[:, 1:FR:2, :], op=A.max,
    )
    # T stage with the TINY floor folded in: t3 >= TINY propagates to
    # every c5 and m5 value, making the final is_ge reject x <= 0.
    e.scalar_tensor_tensor(
        out=t3[:, :, :], in0=p2[:, 0:9, :], scalar=TINY,
        in1=p2[:, 1:10, :], op0=A.max, op1=A.max,
    )
    e.tensor_tensor(
        out=c5[:, 0:ROWS:2, :], in0=t3[:, 0:8, :],
        in1=xb[:, 4:FR:2, :], op=A.max,
    )
    e.tensor_tensor(
        out=c5[:, 1:ROWS:2, :], in0=t3[:, 1:9, :],
        in1=xb[:, 1 : ROWS + 1 : 2, :], op=A.max,
    )
    # --- H pass: 5-max across cols of c5 ---
    e.tensor_tensor(
        out=p1[:, :, 0:hp], in0=c5[:, :, 0:FC:2], in1=c5[:, :, 1:FC:2],
        op=A.max,
    )
    e.tensor_tensor(
        out=tt[:, :, 0 : hp - 1], in0=p1[:, :, 0 : hp - 1],
        in1=p1[:, :, 1:hp], op=A.max,
    )
    e.tensor_tensor(
        out=m5e[:, :, 0:ne], in0=tt[:, :, 0:ne],
        in1=c5[:, :, 4 : 4 + 2 * ne : 2], op=A.max,
    )
    e.tensor_tensor(
        out=m5o[:, :, 0:ne], in0=c5[:, :, 1 : 1 + 2 * ne : 2],
        in1=tt[:, :, 1 : 1 + ne], op=A.max,
    )
    # --- compare: mask = (x >= m5), u8 out ---
    cs = WT * s
    e.tensor_tensor(
        out=msk[:, :, cs : cs + 2 * ne : 2],
        in0=xb[:, 2 : 2 + ROWS, 2 : 2 + 2 * ne : 2],
        in1=m5e[:, :, 0:ne], op=A.is_ge,
    )
    e.tensor_tensor(
        out=msk[:, :, cs + 1 : cs + 2 * ne : 2],
        in0=xb[:, 2 : 2 + ROWS, 3 : 2 + 2 * ne : 2],
        in1=m5o[:, :, 0:ne], op=A.is_ge,
    )


def _build():
    nc = bass.Bass()
    x_in = nc.declare_dram_parameter("scores", [H, W], f32, isOutput=False)
    m_out = nc.declare_dram_parameter("mask", [H, W], u8, isOutput=True)

    from contextlib import ExitStack

    with ExitStack() as stack:
        ec = stack.enter_context
        xb0 = ec(nc.sbuf_tensor("xb0", [128, FR, FC], f32))
        xb1 = ec(nc.sbuf_tensor("xb1", [128, FR, FC], f32))
        xb2 = ec(nc.sbuf_tensor("xb2", [128, FR, FC], f32))
        p2d = ec(nc.sbuf_tensor("p2d", [128, FR // 2, FC], f32))
        t3d = ec(nc.sbuf_tensor("t3d", [128, 9, FC], f32))
        c5d = ec(nc.sbuf_tensor("c5d", [128, ROWS, FC], f32))
        p1d = ec(nc.sbuf_tensor("p1d", [128, ROWS, FC // 2], f32))
        ttd = ec(nc.sbuf_tensor("ttd", [128, ROWS, FC // 2], f32))
        m5ed = ec(nc.sbuf_tensor("m5ed", [128, ROWS, WT // 2], f32))
        m5od = ec(nc.sbuf_tensor("m5od", [128, ROWS, WT // 2], f32))
        msk = ec(nc.sbuf_tensor("msk", [128, ROWS, W], u8))
        block = ec(nc.Block())
        load_sem = ec(nc.semaphore("load_sem"))
        init_sem = ec(nc.semaphore("init_sem"))
        dve_sem = ec(nc.semaphore("dve_sem"))
        out_sem = ec(nc.semaphore("out_sem"))

        bufs = [xb0, xb1, xb2]
        buf_of = lambda s: bufs[2] if s == NSTRIP - 1 else bufs[s % 2]
        td = {"p2": p2d, "t3": t3d, "c5": c5d, "p1": p1d, "tt": ttd,
              "m5e": m5ed, "m5o": m5od}

        def strip_src(s):
            # frame col f = image col WT*s - 2 + f
            c0 = WT * s - 2
            dc = max(0, -c0)           # dst col offset
            c0 = max(0, c0)
            c1 = min(W, WT * s - 2 + FC)
            return c0, dc, c1 - c0

        @block.sync
        def _(sync):
            sync.wait_ge(init_sem, 1)
            for s in range(NSTRIP):
                xb = buf_of(s)
                if s >= 2 and s != NSTRIP - 1:
                    sync.wait_ge(dve_sem, s - 1)
                c0, dc, cw = strip_src(s)
                # partitions 1..126: rows 16p-2 .. 16p+17
                sync.dma_start(
                    out=xb[1:127, :, dc : dc + cw],
                    in_=_dram_ap(
                        x_in, 14 * W + c0, [[16 * W, 126], [W, FR], [1, cw]]
                    ),
                ).then_inc(load_sem, 16)
                # partition 0: rows 0..17 -> frame rows 2..19
                sync.dma_start(
                    out=xb[0:1, 2:FR, dc : dc + cw],
                    in_=_dram_ap(x_in, c0, [[0, 1], [W, FR - 2], [1, cw]]),
                ).then_inc(load_sem, 16)
                # partition 127: rows 2030..2047 -> frame rows 0..17
                sync.dma_start(
                    out=xb[127:128, 0 : FR - 2, dc : dc + cw],
                    in_=_dram_ap(
                        x_in, 2030 * W + c0, [[0, 1], [W, FR - 2], [1, cw]]
                    ),
                ).then_inc(load_sem, 16)

        @block.vector
        def _(v):
            for s in range(NSTRIP):
                v.wait_ge(load_sem, 48 * (s + 1))
                _emit_nms(v, buf_of(s), td, msk, s)
                v.drain().then_inc(dve_sem, 1)

        @block.gpsimd
        def _(g):
            # one-time zero of halo regions never written by loads; wide
            # partition ranges keep accesses quadrant-aligned — the extra
            # partitions' copies are overwritten by every strip load
            for xb in bufs:
                g.memset(xb[0:16, 0:2, :], 0.0)          # p0 top halo rows
                g.memset(xb[96:128, FR - 2 : FR, :], 0.0)  # p127 bottom halo
            g.memset(bufs[0][:, :, 0:2], 0.0)            # strip 0 left halo
            g.memset(bufs[2][:, :, FC - 2 : FC], 0.0)    # last strip right
            g.drain().then_inc(init_sem, 1)

        @block.scalar
        def _(sc):
            # stream mask strips out as compute finishes them
            for s in range(NSTRIP):
                sc.wait_ge(dve_sem, s + 1)
                sc.dma_start(
                    out=_dram_ap(
                        m_out, WT * s, [[16 * W, 128], [W, ROWS], [1, WT]]
                    ),
                    in_=msk[:, :, WT * s : WT * (s + 1)],
                ).then_inc(out_sem, 16)
            sc.wait_ge(out_sem, 16 * NSTRIP)

    return nc


_nc = None


def kernel(scores: np.ndarray) -> np.ndarray:
    global _nc
    scores = np.ascontiguousarray(np.asarray(scores), dtype=np.float32)
    assert scores.shape == (B, 1, H, W), scores.shape
    if _nc is None:
        _nc = _build()
    in_maps = [
        {"scores": np.ascontiguousarray(scores[b, 0])} for b in range(NCORES)
    ]
    res = run_bass_kernel_spmd(_nc, in_maps, list(range(NCORES)), trace=False)
    hs, ws = [], []
    for b in range(NCORES):
        mask = res.results[b]["mask"]
        idx = np.flatnonzero(mask)  # row-major == (h, w) lexicographic
        hs.append((idx // W).astype(np.int32))
        ws.append((idx % W).astype(np.int32))
    hh = np.concatenate(hs)
    ww = np.concatenate(ws)
    n = min(len(hh), MAX_KEYPOINTS)
    out = np.full((2, MAX_KEYPOINTS), -1, dtype=np.int32)
    out[0, :n] = hh[:n]
    out[1, :n] = ww[:n]
    return out


if __name__ == "__main__":
    rng = np.random.default_rng(0)
    x = rng.standard_normal((B, 1, H, W), dtype=np.float32)
    out = kernel(scores=x)
    print("out", out.shape, out.dtype, "nvalid:", int((out[0] >= 0).sum()))


# revision 12
# speedup vs baseline: 1.9776x; 1.0068x over previous
"""NonMaxSuppression (5x5 local max, thr=0) on 8 trn2 NeuronCores — bf16
candidate mask on device at 2x DVE rate + exact fp32 tie resolution on
the host during unsharding.

Input : scores [8, 1, 2048, 2048] fp32 (full).
Output: [2, 2_000_000] int32 — (h, w) coords of survivors in global
        row-major order, padded with -1 (matches jnp.nonzero(size=...)).

Sharding: image b -> core b. The host shards each image into bf16
even/odd COLUMN PLANES (monotone truncation of the fp32 bit pattern),
the device computes the dense 5x5-max candidate mask over the bf16
field, and the host resolves bf16 ties exactly against the fp32 scores
it already holds while unsharding (a candidate is kept iff its fp32
value is the max of its 5x5 window and > 0).

Why bf16 + planes: DVE tensor_tensor runs at 2 elem/cycle for 2-byte
dtypes when every operand's last dim is stride +-1 (measured: 4392 ns
vs 8620 ns fp32 for 8128 elems; arbitrary element offsets are fine,
stride-2 is not). Splitting columns into even/odd planes turns every
shift of the 5-wide window pyramid into a stride-1 access:
  pair    p1[k] = max(E[k], O[k])            (image cols 2k, 2k+1)
  quad    tt[k] = max(p1[k], p1[k+1])        (image cols 2k..2k+3)
  m5 even[2k]   = max(tt[k-1], E[k+1])
  m5 odd [2k+1] = max(O[k-1], tt[k])
The V pass (5-max down rows) is stride-1 in the last dim by
construction. The compare writes uint16 0/1 (u8 output would drop the
compare to 1x).

Correctness: truncation fp32->bf16 is monotone, so a true fp32 window
max always ties the bf16 window max -> the device mask is a SUPERSET
of the true mask; only bf16 ties (~3% of candidates) are pruned by the
host's exact per-candidate check. The final output is bit-exact vs the
fp32 reference.
"""
import sys

sys.path.insert(0, "/opt/trn_rl_repo")
import numpy as np
import ml_dtypes

import concourse.bass as bass
from concourse import mybir
from concourse.bass_utils import run_bass_kernel_spmd

B, H, W = 8, 2048, 2048
NCORES = 8
MAX_KEYPOINTS = 2_000_000

P = W // 2         # plane cols (1024)
ROWS = 16          # image rows per partition (128 * 16 = 2048)
FR = ROWS + 4      # frame rows incl. 2-row halo each side
WTP = 256          # strip width in plane cols (= 512 image cols)
FCP = WTP + 2      # frame cols incl. 1-plane-col halo each side
NSTRIP = P // WTP  # 4

bf16 = mybir.dt.bfloat16
u16 = mybir.dt.uint16


def _dram_ap(t, offset, pattern):
    return bass.AP(tensor=t, offset=offset, ap=pattern)


def _build():
    nc = bass.Bass()
    xe_in = nc.declare_dram_parameter("xe", [H, P], bf16, isOutput=False)
    xo_in = nc.declare_dram_parameter("xo", [H, P], bf16, isOutput=False)
    me_out = nc.declare_dram_parameter("me", [H, P], u16, isOutput=True)
    mo_out = nc.declare_dram_parameter("mo", [H, P], u16, isOutput=True)

    from contextlib import ExitStack

    with ExitStack() as stack:
        ec = stack.enter_context
        xeb = [ec(nc.sbuf_tensor(f"xeb{i}", [128, FR, FCP], bf16))
               for i in range(3)]
        xob = [ec(nc.sbuf_tensor(f"xob{i}", [128, FR, FCP], bf16))
               for i in range(3)]
        p2 = ec(nc.sbuf_tensor("p2", [128, FR // 2, FCP], bf16))
        t3 = ec(nc.sbuf_tensor("t3", [128, 9, FCP], bf16))
        c5e = ec(nc.sbuf_tensor("c5e", [128, ROWS, FCP], bf16))
        c5o = ec(nc.sbuf_tensor("c5o", [128, ROWS, FCP], bf16))
        p1 = ec(nc.sbuf_tensor("p1", [128, ROWS, FCP], bf16))
        tt = ec(nc.sbuf_tensor("tt", [128, ROWS, FCP], bf16))
        m5e = ec(nc.sbuf_tensor("m5e", [128, ROWS, WTP], bf16))
        m5o = ec(nc.sbuf_tensor("m5o", [128, ROWS, WTP], bf16))
        mske = ec(nc.sbuf_tensor("mske", [128, ROWS, P], u16))
        msko = ec(nc.sbuf_tensor("msko", [128, ROWS, P], u16))
        block = ec(nc.Block(no_gpsimd_drain=True))
        load_sem = ec(nc.semaphore("load_sem"))
        dve_sem = ec(nc.semaphore("dve_sem"))
        out_sem = ec(nc.semaphore("out_sem"))

        buf_of = lambda bufs, s: bufs[2] if s == NSTRIP - 1 else bufs[s % 2]

        def strip_src(s):
            # frame col l = plane col WTP*s - 1 + l
            c0 = WTP * s - 1
            dc = max(0, -c0)           # dst col offset
            c0 = max(0, c0)
            c1 = min(P, WTP * s - 1 + FCP)
            return c0, dc, c1 - c0

        @block.sync
        def _(sync):
            for s in range(NSTRIP):
                if s >= 2 and s != NSTRIP - 1:
                    sync.wait_ge(dve_sem, 2 * (s - 1))
                c0, dc, cw = strip_src(s)
                for x_in, bufs in ((xe_in, xeb), (xo_in, xob)):
                    xb = buf_of(bufs, s)
                    # partitions 1..126: rows 16p-2 .. 16p+17
                    sync.dma_start(
                        out=xb[1:127, :, dc : dc + cw],
                        in_=_dram_ap(
                            x_in, 14 * P + c0,
                            [[16 * P, 126], [P, FR], [1, cw]],
                        ),
                    ).then_inc(load_sem, 16)
                    # partition 0: rows 0..17 -> frame rows 2..19, and
                    # image row 0 duplicated into the top halo rows 0..1
                    # (max over a clamped window == max over the true
                    # window for every in-image candidate)
                    sync.dma_start(
                        out=xb[0:1, 2:FR, dc : dc + cw],
                        in_=_dram_ap(x_in, c0, [[0, 1], [P, FR - 2], [1, cw]]),
                    ).then_inc(load_sem, 16)
                    sync.dma_start(
                        out=xb[0:1, 0:2, dc : dc + cw],
                        in_=_dram_ap(x_in, c0, [[0, 1], [0, 2], [1, cw]]),
                    ).then_inc(load_sem, 16)
                    # partition 127: rows 2030..2047 -> frame rows 0..17,
                    # and image row 2047 duplicated into rows 18..19
                    sync.dma_start(
                        out=xb[127:128, 0 : FR - 2, dc : dc + cw],
                        in_=_dram_ap(
                            x_in, 2030 * P + c0,
                            [[0, 1], [P, FR - 2], [1, cw]],
                        ),
                    ).then_inc(load_sem, 16)
                    sync.dma_start(
                        out=xb[127:128, FR - 2 : FR, dc : dc + cw],
                        in_=_dram_ap(
                            x_in, 2047 * P + c0, [[0, 1], [0, 2], [1, cw]]
                        ),
                    ).then_inc(load_sem, 16)

        @block.vector
        def _(v):
            A = mybir.AluOpType
            # Zero only the strip-0 left / strip-3 right halo columns —
            # tiny, disjoint from every load (loads write cols >= dc), and
            # ordered before strip-0 compute by the engine stream. Row
            # halos are filled by duplicate-row DMAs instead.
            v.memset(xeb[0][:, :, 0:1], 0.0)
            v.memset(xob[0][:, :, 0:1], 0.0)
            v.memset(xeb[2][:, :, FCP - 1 : FCP], 0.0)
            v.memset(xob[2][:, :, FCP - 1 : FCP], 0.0)
            for s in range(NSTRIP):
                xe = buf_of(xeb, s)
                xo = buf_of(xob, s)
                # --- V pass per plane: 5-max down rows ---
                # (E-plane loads land first on the queue; start early)
                for xq, c5, lw in ((xe, c5e, 80), (xo, c5o, 160)):
                    v.wait_ge(load_sem, 160 * s + lw)
                    v.tensor_tensor(
                        out=p2[:, :, :], in0=xq[:, 0:FR:2, :],
                        in1=xq[:, 1:FR:2, :], op=A.max,
                    )
                    v.tensor_tensor(
                        out=t3[:, :, :], in0=p2[:, 0:9, :],
                        in1=p2[:, 1:10, :], op=A.max,
                    )
                    v.tensor_tensor(
                        out=c5[:, 0:ROWS:2, :], in0=t3[:, 0:8, :],
                        in1=xq[:, 4:FR:2, :], op=A.max,
                    )
                    v.tensor_tensor(
                        out=c5[:, 1:ROWS:2, :], in0=t3[:, 1:9, :],
                        in1=xq[:, 1 : ROWS + 1 : 2, :], op=A.max,
                    )
                # --- H pass across planes ---
                v.tensor_tensor(
                    out=p1[:, :, :], in0=c5e[:, :, :], in1=c5o[:, :, :],
                    op=A.max,
                )
                v.tensor_tensor(
                    out=tt[:, :, 0 : FCP - 1], in0=p1[:, :, 0 : FCP - 1],
                    in1=p1[:, :, 1:FCP], op=A.max,
                )
                # even plane finishes first so its store overlaps the odd
                # plane's remaining compute; the last strip splits each
                # compare in half so the final store tail is smaller
                cs = WTP * s
                hw_ = WTP // 2
                halves = 2 if s == NSTRIP - 1 else 1
                v.tensor_tensor(
                    out=m5e[:, :, :], in0=tt[:, :, 0:WTP],
                    in1=c5e[:, :, 2:FCP], op=A.max,
                )
                for hh in range(halves):
                    o, n = (hh * hw_, hw_) if halves == 2 else (0, WTP)
                    v.tensor_tensor(
                        out=mske[:, :, cs + o : cs + o + n],
                        in0=xe[:, 2 : 2 + ROWS, 1 + o : 1 + o + n],
                        in1=m5e[:, :, o : o + n], op=A.is_ge,
                    ).then_inc(dve_sem, 1)
                v.tensor_tensor(
                    out=m5o[:, :, :], in0=c5o[:, :, 0:WTP],
                    in1=tt[:, :, 1 : WTP + 1], op=A.max,
                )
                for hh in range(halves):
                    o, n = (hh * hw_, hw_) if halves == 2 else (0, WTP)
                    v.tensor_tensor(
                        out=msko[:, :, cs + o : cs + o + n],
                        in0=xo[:, 2 : 2 + ROWS, 1 + o : 1 + o + n],
                        in1=m5o[:, :, o : o + n], op=A.is_ge,
                    ).then_inc(dve_sem, 1)

        @block.scalar
        def _(sc):
            done = 0
            for s in range(NSTRIP):
                cs = WTP * s
                hw_ = WTP // 2
                halves = 2 if s == NSTRIP - 1 else 1
                for m_out, mbuf in ((me_out, mske), (mo_out, msko)):
                    for hh in range(halves):
                        o = hh * hw_ if halves == 2 else 0
                        n = hw_ if halves == 2 else WTP
                        done += 1
                        sc.wait_ge(dve_sem, done)
                        sc.dma_start(
                            out=_dram_ap(
                                m_out, cs + o,
                                [[16 * P, 128], [P, ROWS], [1, n]],
                            ),
                            in_=mbuf[:, :, cs + o : cs + o + n],
                        ).then_inc(out_sem, 16)
            sc.wait_ge(out_sem, 16 * done)

    return nc


_nc = None

_DH, _DW = np.meshgrid(np.arange(5), np.arange(5), indexing="ij")
_DH = _DH.ravel()
_DW = _DW.ravel()


def _resolve(img, me, mo):
    """Exact fp32 verification of the bf16 candidate mask for one image.

    Returns (hs, ws) int32 arrays in row-major order."""
    cand = np.zeros((H, W), dtype=bool)
    cand[:, 0::2] = me != 0
    cand[:, 1::2] = mo != 0
    hs, ws = np.nonzero(cand)
    x = img[hs, ws]
    pad = np.full((H + 4, W + 4), -np.inf, dtype=np.float32)
    pad[2 : 2 + H, 2 : 2 + W] = img
    mx = np.full(x.shape, -np.inf, dtype=np.float32)
    for dh, dw in zip(_DH, _DW):
        np.maximum(mx, pad[hs + dh, ws + dw], out=mx)
    keep = (x > 0.0) & (x >= mx)   # x in window => x >= mx iff x == max
    return hs[keep].astype(np.int32), ws[keep].astype(np.int32)


def kernel(scores: np.ndarray) -> np.ndarray:
    global _nc
    scores = np.ascontiguousarray(np.asarray(scores), dtype=np.float32)
    assert scores.shape == (B, 1, H, W), scores.shape
    if _nc is None:
        _nc = _build()
    imgs = [np.ascontiguousarray(scores[b, 0]) for b in range(NCORES)]
    in_maps = []
    for img in imgs:
        hi = (img.view(np.uint32) >> 16).astype(np.uint16)  # bf16 trunc
        in_maps.append({
            "xe": np.ascontiguousarray(hi[:, 0::2]).view(ml_dtypes.bfloat16),
            "xo": np.ascontiguousarray(hi[:, 1::2]).view(ml_dtypes.bfloat16),
        })
    res = run_bass_kernel_spmd(_nc, in_maps, list(range(NCORES)), trace=False)
    hs, ws = [], []
    for b in range(NCORES):
        hb, wb = _resolve(
            imgs[b],
            np.asarray(res.results[b]["me"]),
            np.asarray(res.results[b]["mo"]),
        )
        hs.append(hb)
        ws.append(wb)
    hh = np.concatenate(hs)
    ww = np.concatenate(ws)
    n = min(len(hh), MAX_KEYPOINTS)
    out = np.full((2, MAX_KEYPOINTS), -1, dtype=np.int32)
    out[0, :n] = hh[:n]
    out[1, :n] = ww[:n]
    return out


if __name__ == "__main__":
    rng = np.random.default_rng(0)
    x = rng.standard_normal((B, 1, H, W), dtype=np.float32)
    out = kernel(scores=x)
    print("out", out.shape, out.dtype, "nvalid:", int((out[0] >= 0).sum()))
